# revision 1
# baseline (speedup 1.0000x reference)
"""Trainium2 Bass kernel for nn_KMLoss (segment_reduce proto-network loss).

Math (exact decomposition of the reference):
  logit[q,s] = -0.5*sq(q,s), sq = |xq_q - xs_s|^2 (the reference's clamp at 0
  never fires for this data: min sq ~ 330).  Let L = 0.5*sq >= 0 (logit = -L).

  neg path: per-class column sums of logit are LINEAR in logit, so
    G[q,c] = sum_{s in c} L[q,s] = 0.5*cnt_c*|xq_q|^2 + 0.5*S2_c - xq_q . T_c
  with T_c = sum_{s in c} xs_s, S2_c = sum_{s in c} |xs_s|^2 -> tiny GEMM
  against per-class aggregates.  A = G/adj = -normalized (1/adj folded into
  host-prescaled GEMM columns; the query's own class is reordered to column 0
  so a single [128,1] tensor_scalar applies the self-correction).
  neg = LSE_c(normalized) = ln(sum exp(Mn2 - A)) - Mn2,  Mn2 = min(A).

  pos path: logsumexp over same-class support columns only -> block-diagonal
  [nq_c x ns_c] blocks once queries and support are grouped by class (~1/64
  of the full [Nq,Ns] logit matrix).  Self column pushed out via +2000 mask.

Sharding: core k owns classes [8k, 8k+8); its queries are the queries of
those classes (padded to 128 rows/class -> 8 uniform blocks per core, so the
SPMD program is identical across cores; only input data differs).  Per block
one matmul group computes [128, W+64] = [pos-block | A-block] in PSUM.
Device outputs per-row (min, sum-exp) for both paths; host takes logs and
means.  All DMAs are whole-tensor contiguous (8 loads + 1 store per core).
"""

import sys

import numpy as np

sys.path.insert(0, "/opt/trn_rl_repo")

NCORES = 8
C = 64
CPB = C // NCORES  # classes per core == blocks per core
QC = 128  # padded query rows per class
D = 256
KAUG = 2  # fp32 augmented contraction rows: [0.5*|xq|^2 ; 1]
INF = 1000.0

_PROGRAM_CACHE = {}


def _build_program(W):
    """Build the SPMD-uniform Bass program for class-block width W."""
    import concourse.bacc as bacc
    import concourse.tile as tile
    from concourse import mybir

    dt = mybir.dt
    Alu = mybir.AluOpType
    Act = mybir.ActivationFunctionType
    NCOL = W + C

    nc = bacc.Bacc(
        "TRN2",
        target_bir_lowering=False,
        debug=False,
        enable_asserts=False,
        num_devices=NCORES,
    )

    lhs0 = nc.dram_tensor("lhs0", [128, CPB * QC], dt.bfloat16, kind="ExternalInput").ap()
    lhs1 = nc.dram_tensor("lhs1", [128, CPB * QC], dt.bfloat16, kind="ExternalInput").ap()
    lhs2 = nc.dram_tensor("lhs2", [KAUG, CPB * QC], dt.float32, kind="ExternalInput").ap()
    rhs0 = nc.dram_tensor("rhs0", [128, CPB * NCOL], dt.bfloat16, kind="ExternalInput").ap()
    rhs1 = nc.dram_tensor("rhs1", [128, CPB * NCOL], dt.bfloat16, kind="ExternalInput").ap()
    rhs2 = nc.dram_tensor("rhs2", [KAUG, CPB * NCOL], dt.float32, kind="ExternalInput").ap()
    mask = nc.dram_tensor("mask", [128, CPB * W], dt.bfloat16, kind="ExternalInput").ap()
    corrdiv = nc.dram_tensor("corrdiv", [128, CPB], dt.float32, kind="ExternalInput").ap()
    out = nc.dram_tensor("out", [128, 4 * CPB], dt.float32, kind="ExternalOutput").ap()

    with tile.TileContext(nc) as tc:
        with (
            tc.tile_pool(name="io", bufs=1) as io,
            tc.tile_pool(name="work", bufs=3) as work,
            tc.tile_pool(name="pp", bufs=4, space="PSUM") as pp,
        ):
            s_l0 = io.tile([128, CPB * QC], dt.bfloat16)
            nc.sync.dma_start(out=s_l0, in_=lhs0)
            s_l1 = io.tile([128, CPB * QC], dt.bfloat16)
            nc.sync.dma_start(out=s_l1, in_=lhs1)
            s_l2 = io.tile([KAUG, CPB * QC], dt.float32)
            nc.sync.dma_start(out=s_l2, in_=lhs2)
            s_r0 = io.tile([128, CPB * NCOL], dt.bfloat16)
            nc.sync.dma_start(out=s_r0, in_=rhs0)
            s_r1 = io.tile([128, CPB * NCOL], dt.bfloat16)
            nc.sync.dma_start(out=s_r1, in_=rhs1)
            s_r2 = io.tile([KAUG, CPB * NCOL], dt.float32)
            nc.sync.dma_start(out=s_r2, in_=rhs2)
            s_mk = io.tile([128, CPB * W], dt.bfloat16)
            nc.sync.dma_start(out=s_mk, in_=mask)
            s_cd = io.tile([128, CPB], dt.float32)
            nc.sync.dma_start(out=s_cd, in_=corrdiv)

            # packed output: [Sn | Mn2 | S | Mn] each [128, CPB]
            outt = io.tile([128, 4 * CPB], dt.float32)
            Snall = outt[:, 0:CPB]
            Mn2all = outt[:, CPB:2 * CPB]
            Sall = outt[:, 2 * CPB:3 * CPB]
            Mnall = outt[:, 3 * CPB:4 * CPB]

            for b in range(CPB):
                qs = slice(b * QC, (b + 1) * QC)
                cs = slice(b * NCOL, (b + 1) * NCOL)
                ps = pp.tile([128, NCOL], dt.float32)
                nc.tensor.matmul(ps, s_l0[:, qs], s_r0[:, cs], start=True, stop=False)
                nc.tensor.matmul(ps, s_l1[:, qs], s_r1[:, cs], start=False, stop=False)
                nc.tensor.matmul(ps, s_l2[:, qs], s_r2[:, cs], start=False, stop=True)

                # neg path: own-class column (WG col 0) self-correction, then
                # Mn2 = min(A), Sn = sum exp(Mn2 - A)  over A = ps[:, W:]
                nc.vector.tensor_scalar_sub(
                    out=ps[:, W:W + 1], in0=ps[:, W:W + 1], scalar1=s_cd[:, b:b + 1]
                )
                nc.vector.tensor_reduce(
                    out=Mn2all[:, b:b + 1], in_=ps[:, W:NCOL],
                    axis=mybir.AxisListType.X, op=Alu.min,
                )
                En = work.tile([128, C], dt.float32, tag="En")
                nc.scalar.activation(
                    En, ps[:, W:NCOL], Act.Exp, bias=Mn2all[:, b:b + 1],
                    scale=-1.0, accum_out=Snall[:, b:b + 1],
                )

                # pos path: P2 = L + mask ; Mn = min ; S = sum exp(Mn - P2)
                P2 = work.tile([128, W], dt.float32, tag="P2")
                nc.vector.tensor_tensor(
                    out=P2, in0=ps[:, 0:W], in1=s_mk[:, b * W:(b + 1) * W], op=Alu.add
                )
                nc.vector.tensor_reduce(
                    out=Mnall[:, b:b + 1], in_=P2,
                    axis=mybir.AxisListType.X, op=Alu.min,
                )
                E = work.tile([128, W], dt.float32, tag="E")
                nc.scalar.activation(
                    E, P2, Act.Exp, bias=Mnall[:, b:b + 1], scale=-1.0,
                    accum_out=Sall[:, b:b + 1],
                )

            nc.sync.dma_start(out=out, in_=outt)

    nc.compile()
    return nc


def _prepare(xq, yq, xs, ys, pos):
    """Host-side prep: class grouping, aggregates, per-core input arrays."""
    import ml_dtypes

    bf16 = ml_dtypes.bfloat16
    Nq = xq.shape[0]
    xq64 = xq.astype(np.float64)
    xs64 = xs.astype(np.float64)

    cnt = np.bincount(ys, minlength=C).astype(np.float64)
    T_c = np.zeros((C, D), np.float64)
    np.add.at(T_c, ys, xs64)
    S2_c = np.zeros(C, np.float64)
    np.add.at(S2_c, ys, (xs64 ** 2).sum(-1))
    S2_eff = np.where(cnt > 0, S2_c, 4e6)  # empty class -> huge A -> excluded

    xq2 = (xq64 ** 2).sum(-1)
    xs2 = (xs64 ** 2).sum(-1)

    sidx = [np.where(ys == c)[0] for c in range(C)]
    qidx = [np.where(yq == c)[0] for c in range(C)]
    max_ns = max(1, max(len(s) for s in sidx))
    max_nq = max(len(q) for q in qidx)
    assert max_nq <= QC, f"class query count {max_nq} exceeds {QC}"
    W = -(-max_ns // 16) * 16
    NCOL = W + C

    xs_twin = xs64[pos]
    L_self = 0.5 * ((xq64 - xs_twin) ** 2).sum(-1)

    in_maps = []
    meta = []
    for k in range(NCORES):
        lhs_dot = np.zeros((D, CPB * QC), np.float32)
        lhs_aug = np.zeros((KAUG, CPB * QC), np.float32)
        rhs_dot = np.zeros((D, CPB * NCOL), np.float32)
        rhs_aug = np.zeros((KAUG, CPB * NCOL), np.float32)
        mk = np.zeros((128, CPB * W), np.float32)
        cd = np.zeros((128, CPB), np.float32)
        core_meta = []
        for b in range(CPB):
            cb = k * CPB + b
            qi = qidx[cb]
            si = sidx[cb]
            nq, ns = len(qi), len(si)
            qs = slice(b * QC, b * QC + nq)
            lhs_dot[:, qs] = -xq[qi].T
            lhs_aug[0, qs] = (0.5 * xq2[qi]).astype(np.float32)
            lhs_aug[1, qs] = 1.0
            # pos columns
            ss = slice(b * NCOL, b * NCOL + ns)
            rhs_dot[:, ss] = xs[si].T
            rhs_aug[0, ss] = 1.0
            rhs_aug[1, ss] = (0.5 * xs2[si]).astype(np.float32)
            # pad pos columns: L_pad = 0.5*xq2 + 2000
            ps_ = slice(b * NCOL + ns, b * NCOL + W)
            rhs_aug[0, ps_] = 1.0
            rhs_aug[1, ps_] = 2000.0
            # WG columns, own class first, scaled by 1/adj
            order = [cb] + [c for c in range(C) if c != cb]
            adj = np.array(
                [cnt[c] - (1.0 if c == cb else 0.0) for c in order], np.float64
            )
            s_j = 1.0 / np.maximum(adj, 1.0)
            s_j[adj <= 0] = 1.0
            ocols = np.array(order)
            gs = slice(b * NCOL + W, (b + 1) * NCOL)
            rhs_dot[:, gs] = (T_c[ocols].T * s_j[None, :]).astype(np.float32)
            rhs_aug[0, gs] = (cnt[ocols] * s_j).astype(np.float32)
            rhs_aug[1, gs] = (0.5 * S2_eff[ocols] * s_j).astype(np.float32)
            if nq:
                selfpos = np.searchsorted(si, pos[qi])
                assert ns and (si[selfpos] == pos[qi]).all(), \
                    "pos[q] must be same-class support"
                r = np.arange(nq)
                single = cnt[cb] <= 1
                mk[r, b * W + selfpos] = np.where(
                    single, -L_self[qi], 2000.0
                ).astype(np.float32)
                corr = L_self[qi] - INF * (cnt[cb] > 1)
                cd[:nq, b] = (corr * s_j[0]).astype(np.float32)
            core_meta.append((cb, qi))
        in_maps.append({
            "lhs0": lhs_dot[0:128].astype(bf16),
            "lhs1": lhs_dot[128:256].astype(bf16),
            "lhs2": lhs_aug,
            "rhs0": rhs_dot[0:128].astype(bf16),
            "rhs1": rhs_dot[128:256].astype(bf16),
            "rhs2": rhs_aug,
            "mask": mk.astype(bf16),
            "corrdiv": cd,
        })
        meta.append(core_meta)
    return W, in_maps, meta, Nq


def _reduce_host(results, meta, Nq):
    total = 0.0
    for k in range(NCORES):
        o = np.asarray(results[k]["out"], np.float64)
        Sn, Mn2 = o[:, 0:CPB], o[:, CPB:2 * CPB]
        S, Mn = o[:, 2 * CPB:3 * CPB], o[:, 3 * CPB:4 * CPB]
        neg = np.log(Sn) - Mn2
        pos = np.log(S) - Mn
        for b, (cb, qi) in enumerate(meta[k]):
            n = len(qi)
            if n:
                total += (neg[:n, b] - pos[:n, b]).sum()
    return np.array(total / Nq, dtype=np.float32)


def _run(xq, yq, xs, ys, pos, trace=False, tmpdir=None):
    from concourse import bass_utils

    xq = np.ascontiguousarray(np.asarray(xq, np.float32))
    xs = np.ascontiguousarray(np.asarray(xs, np.float32))
    yq = np.asarray(yq).astype(np.int64)
    ys = np.asarray(ys).astype(np.int64)
    pos = np.asarray(pos).astype(np.int64)

    W, in_maps, meta, Nq = _prepare(xq, yq, xs, ys, pos)
    if W not in _PROGRAM_CACHE:
        _PROGRAM_CACHE[W] = _build_program(W)
    nc = _PROGRAM_CACHE[W]

    kw = {}
    if trace:
        kw = dict(trace=True, tmpdir=tmpdir)
    res = bass_utils.run_bass_kernel_spmd(
        nc, in_maps, core_ids=list(range(NCORES)), **kw
    )
    return _reduce_host(res.results, meta, Nq), res


def kernel(xq, yq, xs, ys, pos):
    loss, _ = _run(xq, yq, xs, ys, pos, trace=False)
    return loss

